# revision 1
# baseline (speedup 1.0000x reference)
"""Bass/Trainium2 kernel for nn_BDHAttentionLayer (B=2, S=2048, DM=1024, H=16).

ReLU-attention layer: Q/K/V projections, causal relu-normalized attention,
output projection. Sharded over 8 NeuronCores: data-parallel over batch (2)
x head-parallel (16 heads -> 4 heads per core). Each core computes a partial
(DM, S) transposed output for its batch; host sums the 4 head-group partials
per batch, transposes, and adds the output bias.

All on-chip layouts are chosen so no transposes are needed on hardware:
the host feeds x^T and pre-transposed weight slices, attention runs in
(key, query) orientation, and the attention row-sum (the relu-attention
normalizer) falls out of the context matmul via a ones-column appended to V.
The normalizer reciprocal is partition-broadcast with a K=1 matmul.

All matmuls run in float32r (full-rate fp32 PE mode). PSUM evacuations read
PSUM through a float32r bitcast so the writes stay on the fast same-dtype
path while still satisfying the FP32r producer check.
"""

import contextlib

import numpy as np

import concourse.bass as bass
import concourse.mybir as mybir
import concourse.tile as tile
from concourse import bacc
from concourse.bass import ds, ts
from concourse.bass_utils import run_bass_kernel_spmd

B, S, DM, H = 2, 2048, 1024, 16
DH = DM // H
EPS = 1e-9
N_CORES = 8
HPC = H // (N_CORES // B)  # heads per core = 4
DCG = HPC * DH  # hidden dims per core = 256
KO = DM // 128  # contraction tiles for projections = 8
SC = S // 512  # 512-wide s/q chunks = 4
ST = S // 128  # 128-wide s/k tiles = 16
VW = 96  # V tile width: 64 ctx dims + ones col + pad to a 32 multiple
SCALE = 1.0 / np.sqrt(DH)

F32 = mybir.dt.float32
F32R = mybir.dt.float32r
AF = mybir.ActivationFunctionType

_CACHED = {}


def _build(reps=1, phases=3):
    nc = bacc.Bacc("TRN2", debug=False, num_devices=N_CORES)
    xT = nc.dram_tensor("xT", (DM, S), F32R, kind="ExternalInput")
    wq = nc.dram_tensor("wq", (DM, DCG), F32R, kind="ExternalInput")
    wk = nc.dram_tensor("wk", (DM, DCG), F32R, kind="ExternalInput")
    wv = nc.dram_tensor("wv", (DM, DCG), F32R, kind="ExternalInput")
    wo = nc.dram_tensor("wo", (DCG, DM), F32R, kind="ExternalInput")
    bqv = nc.dram_tensor("bqv", (DCG, 1), F32, kind="ExternalInput")
    bkv = nc.dram_tensor("bkv", (DCG, 1), F32, kind="ExternalInput")
    bvv = nc.dram_tensor("bvv", (DCG,), F32R, kind="ExternalInput")
    ones = nc.dram_tensor("ones", (65, 64), F32R, kind="ExternalInput")
    outT = nc.dram_tensor("outT", (DM, S), F32, kind="ExternalOutput")

    with tile.TileContext(nc) as tc:
        with (
            tc.tile_pool(name="consts", bufs=1) as consts,
            tc.tile_pool(name="qkv", bufs=1) as qkv,
        ):
            # ---- constant loads ----
            wq_sb = consts.tile([128, KO, DCG], F32R)
            wk_sb = consts.tile([128, KO, DCG], F32R)
            wv_sb = consts.tile([128, KO, DCG], F32R)
            wo_sb = consts.tile([128, 2, DM], F32R)
            for ko in range(KO):
                nc.sync.dma_start(out=wq_sb[:, ko, :], in_=wq.ap()[ts(ko, 128), :])
                nc.sync.dma_start(out=wk_sb[:, ko, :], in_=wk.ap()[ts(ko, 128), :])
                nc.sync.dma_start(out=wv_sb[:, ko, :], in_=wv.ap()[ts(ko, 128), :])
            for dc in range(2):
                nc.sync.dma_start(out=wo_sb[:, dc, :], in_=wo.ap()[ts(dc, 128), :])
            bq_sb = consts.tile([128, 2, 1], F32)
            bk_sb = consts.tile([128, 2, 1], F32)
            nc.sync.dma_start(
                out=bq_sb, in_=bqv.ap().rearrange("(t p) o -> p t o", p=128)
            )
            nc.sync.dma_start(
                out=bk_sb, in_=bkv.ap().rearrange("(t p) o -> p t o", p=128)
            )
            bv_bc = consts.tile([128, DCG], F32R)
            bvap = bvv.ap()
            nc.sync.dma_start(
                out=bv_bc,
                in_=bass.AP(tensor=bvap.tensor, offset=0, ap=[[0, 128], [1, DCG]]),
            )
            ones64 = consts.tile([65, 64], F32R)
            nc.sync.dma_start(out=ones64, in_=ones.ap())

            q_sb = qkv.tile([128, 2, S], F32R)
            k_sb = qkv.tile([128, 2, S], F32R)
            v_sb = qkv.tile([128, ST, HPC, VW], F32R)
            ctx_sb = qkv.tile([128, 2, S], F32R)

            loop_cm = tc.For_i(0, reps, 1) if reps > 1 else contextlib.nullcontext()
            with loop_cm:
                # ---- phase 1: projections ----
                with (
                    tc.tile_pool(name="xp", bufs=1) as xp,
                    tc.tile_pool(name="ps_proj", bufs=4, space="PSUM") as ps_proj,
                ):
                    x_sb = xp.tile([128, KO, S], F32R)
                    for ko in range(KO):
                        for half in range(2):
                            nc.sync.dma_start(
                                out=x_sb[:, ko, ds(1024 * half, 1024)],
                                in_=xT.ap()[ts(ko, 128), ds(1024 * half, 1024)],
                            )
                    for t in range(2):
                        for j in range(SC):
                            pq = ps_proj.tile([128, 512], F32, tag="pp")
                            for ko in range(KO):
                                nc.tensor.matmul(
                                    pq,
                                    wq_sb[:, ko, ts(t, 128)],
                                    x_sb[:, ko, ds(512 * j, 512)],
                                    start=(ko == 0),
                                    stop=(ko == KO - 1),
                                )
                            nc.scalar.activation(
                                out=q_sb[:, t, ds(512 * j, 512)],
                                in_=pq[:].bitcast(F32R),
                                func=AF.Identity,
                                bias=bq_sb[:, t, 0:1],
                            )
                            pk = ps_proj.tile([128, 512], F32, tag="pp")
                            for ko in range(KO):
                                nc.tensor.matmul(
                                    pk,
                                    wk_sb[:, ko, ts(t, 128)],
                                    x_sb[:, ko, ds(512 * j, 512)],
                                    start=(ko == 0),
                                    stop=(ko == KO - 1),
                                )
                            nc.scalar.activation(
                                out=k_sb[:, t, ds(512 * j, 512)],
                                in_=pk[:].bitcast(F32R),
                                func=AF.Identity,
                                bias=bk_sb[:, t, 0:1],
                            )
                    for st in range(ST):
                        pv = ps_proj.tile([128, DCG], F32, tag="pp")
                        for ko in range(KO):
                            nc.tensor.matmul(
                                pv,
                                x_sb[:, ko, ts(st, 128)],
                                wv_sb[:, ko, :],
                                start=(ko == 0),
                                stop=(ko == KO - 1),
                            )
                        nc.vector.tensor_add(
                            out=v_sb[:, st, :, 0:DH],
                            in0=pv[:]
                            .bitcast(F32R)
                            .rearrange("p (h d) -> p h d", h=HPC),
                            in1=bv_bc[:].rearrange("p (h d) -> p h d", h=HPC),
                        )
                        # ones column (+ finite padding to the 96-col tile)
                        nc.scalar.activation(
                            out=v_sb[:, st, :, DH:VW],
                            in_=pv[:, 0 : HPC * (VW - DH)]
                            .bitcast(F32R)
                            .rearrange("p (h c) -> p h c", h=HPC),
                            func=AF.Identity,
                            scale=0.0,
                            bias=1.0,
                        )

                # ---- phase 2: attention, (k, q) orientation ----
                if phases >= 2:
                    with (
                        tc.tile_pool(name="attn", bufs=8) as attn_p,
                        tc.tile_pool(name="smalls", bufs=3) as smalls,
                        tc.tile_pool(name="ps_score", bufs=5, space="PSUM") as ps_score,
                        tc.tile_pool(name="ps_ctx", bufs=2, space="PSUM") as ps_ctx,
                        tc.tile_pool(name="ps_bc", bufs=1, space="PSUM") as ps_bc,
                    ):
                        for p in range(2):  # heads 2p (base 0), 2p+1 (base 64)
                            for j in range(SC):
                                cps = {}
                                for h in (2 * p, 2 * p + 1):
                                    cph = ps_ctx.tile([VW, 512], F32, tag="ctx")
                                    cps[h] = cph
                                n_k = 4 * j + 4  # causal: k-tiles 0 .. 4j+3
                                pend = []  # ctx MMs lag two i (sw pipeline)
                                for i in range(n_k):
                                    cur = []
                                    c0 = 128 * (i - 4 * j) if i >= 4 * j else 0
                                    for h in (2 * p, 2 * p + 1):
                                        base = 64 * (h % 2)
                                        sps = ps_score.tile(
                                            [128, 512], F32, tag="score"
                                        )
                                        nc.tensor.matmul(
                                            sps[:, c0:512],
                                            k_sb[base : base + 64, p, ts(i, 128)],
                                            q_sb[
                                                base : base + 64,
                                                p,
                                                ds(512 * j + c0, 512 - c0),
                                            ],
                                            start=True,
                                            stop=True,
                                        )
                                        at = attn_p.tile([128, 512], F32R, tag="attn")
                                        if h % 2 == 0:
                                            nc.scalar.activation(
                                                out=at[:, c0:512],
                                                in_=sps[:, c0:512].bitcast(F32R),
                                                func=AF.Relu,
                                            )
                                        else:
                                            nc.vector.tensor_scalar_max(
                                                at[:, c0:512],
                                                sps[:, c0:512].bitcast(F32R),
                                                0.0,
                                            )
                                        if i >= 4 * j:  # diagonal 128-col block
                                            # keep where q >= k:
                                            # (512j + f) - (128i + p) >= 0
                                            nc.gpsimd.affine_select(
                                                out=at[:, c0 : c0 + 128],
                                                in_=at[:, c0 : c0 + 128],
                                                compare_op=mybir.AluOpType.is_ge,
                                                fill=0.0,
                                                base=512 * j + c0 - 128 * i,
                                                channel_multiplier=-1,
                                                pattern=[[1, 128]],
                                            )
                                        cur.append((h, i, at, c0))
                                    pend.append(cur)
                                    if len(pend) > 2:
                                        for (h, ii, at, cc) in pend.pop(0):
                                            nc.tensor.matmul(
                                                cps[h][:VW, cc:512],
                                                v_sb[:, ii, h, :],
                                                at[:, cc:512],
                                                start=(ii == 0),
                                                stop=(ii == n_k - 1),
                                            )
                                for round_ in pend:
                                    for (h, ii, at, cc) in round_:
                                        nc.tensor.matmul(
                                            cps[h][:VW, cc:512],
                                            v_sb[:, ii, h, :],
                                            at[:, cc:512],
                                            start=(ii == 0),
                                            stop=(ii == n_k - 1),
                                        )
                                for h in (2 * p, 2 * p + 1):
                                    base = 64 * (h % 2)
                                    den = smalls.tile([65, 512], F32, tag="den")
                                    nc.vector.tensor_scalar_add(
                                        den[64:65, :], cps[h][64:65, :], EPS
                                    )
                                    # custom-DVE reciprocal needs partition 0
                                    d0 = smalls.tile([1, 512], F32, tag="d0")
                                    nc.sync.dma_start(out=d0, in_=den[64:65, :])
                                    scr = smalls.tile([1, 512], F32, tag="scr")
                                    dr = smalls.tile([1, 512], F32, tag="dr")
                                    nc.vector.reciprocal_approx_accurate(
                                        out=dr, in_=d0, scratch=scr
                                    )
                                    denr = smalls.tile([1, 512], F32R, tag="denr")
                                    nc.sync.dma_start(
                                        out=denr, in_=dr[:].bitcast(F32R)
                                    )
                                    # partition-broadcast 1/den: K=1 matmul
                                    bc = ps_bc.tile([64, 512], F32, tag="bc")
                                    nc.tensor.matmul(
                                        bc,
                                        ones64[0:1, :],
                                        denr,
                                        start=True,
                                        stop=True,
                                    )
                                    tmp = smalls.tile([64, 512], F32, tag="tmp")
                                    if h % 2 == 0:
                                        nc.vector.tensor_copy(tmp, cps[h][0:DH, :])
                                    else:
                                        nc.scalar.activation(
                                            out=tmp, in_=cps[h][0:DH, :], func=AF.Copy
                                        )
                                    tmp2 = smalls.tile([64, 512], F32, tag="tmp2")
                                    nc.vector.tensor_mul(out=tmp2, in0=tmp, in1=bc)
                                    # retag f32->f32r + head-base relocation
                                    nc.sync.dma_start(
                                        out=ctx_sb[
                                            base : base + 64, p, ds(512 * j, 512)
                                        ],
                                        in_=tmp2[:].bitcast(F32R),
                                    )

                # ---- phase 3: output projection (partial) ----
                if phases >= 3:
                    with (
                        tc.tile_pool(name="outs", bufs=4) as outp,
                        tc.tile_pool(name="ps_out", bufs=4, space="PSUM") as ps_out,
                    ):
                        for dt in range(KO):
                            for j in range(SC):
                                po = ps_out.tile([128, 512], F32, tag="po")
                                for dc in range(2):
                                    nc.tensor.matmul(
                                        po,
                                        wo_sb[:, dc, ts(dt, 128)],
                                        ctx_sb[:, dc, ds(512 * j, 512)],
                                        start=(dc == 0),
                                        stop=(dc == 1),
                                    )
                                ot = outp.tile([128, 512], F32, tag="ot")
                                nc.vector.tensor_copy(ot, po)
                                nc.sync.dma_start(
                                    out=outT.ap()[ts(dt, 128), ds(512 * j, 512)],
                                    in_=ot,
                                )
    nc.compile()
    return nc


def _get_nc():
    if "nc" not in _CACHED:
        _CACHED["nc"] = _build()
    return _CACHED["nc"]


def _in_maps(x, Wq, bq, Wk, bk, Wv, bv, Wo):
    xTs = [np.ascontiguousarray(x[b].T) for b in range(B)]
    maps = []
    for c in range(N_CORES):
        b, hg = divmod(c, N_CORES // B)
        hs = slice(hg * DCG, (hg + 1) * DCG)
        maps.append(
            {
                "xT": xTs[b],
                # fold the 1/sqrt(DH) score scale into the Q projection
                "wq": np.ascontiguousarray(Wq[hs].T) * SCALE,
                "wk": np.ascontiguousarray(Wk[hs].T),
                "wv": np.ascontiguousarray(Wv[hs].T),
                "wo": np.ascontiguousarray(Wo[:, hs].T),
                "bqv": (bq[hs] * SCALE).reshape(DCG, 1).astype(np.float32),
                "bkv": bk[hs].reshape(DCG, 1).astype(np.float32),
                "bvv": bv[hs].astype(np.float32),
                "ones": np.ones((65, 64), dtype=np.float32),
            }
        )
    return maps


def kernel(x, Wq, bq, Wk, bk, Wv, bv, Wo, bo, _trace=False):
    x = np.asarray(x, dtype=np.float32)
    Wq, bq = np.asarray(Wq, np.float32), np.asarray(bq, np.float32)
    Wk, bk = np.asarray(Wk, np.float32), np.asarray(bk, np.float32)
    Wv, bv = np.asarray(Wv, np.float32), np.asarray(bv, np.float32)
    Wo, bo = np.asarray(Wo, np.float32), np.asarray(bo, np.float32)

    nc = _get_nc()
    res = run_bass_kernel_spmd(
        nc,
        _in_maps(x, Wq, bq, Wk, bk, Wv, bv, Wo),
        core_ids=list(range(N_CORES)),
        trace=_trace,
    )

    out = np.empty((B, S, DM), dtype=np.float32)
    for b in range(B):
        acc = res.results[b * (N_CORES // B)]["outT"].astype(np.float32)
        for g in range(1, N_CORES // B):
            acc = acc + res.results[b * (N_CORES // B) + g]["outT"]
        out[b] = acc.T + bo
    if _trace:
        return out, res
    return out

